# revision 33
# baseline (speedup 1.0000x reference)
"""Trainium2 8-core Bass kernel for nn_AI4Urban (CFD step + multigrid).

Self-contained: builds per-call (weights/dt baked as compile-time consts),
shards the 128^3 grid along z across 8 NeuronCores with 3-deep ghost input
planes, runs all 3x3x3 stencils as banded f32r matmuls on the PE
(x in partitions, (z,y) in the free dim), does the multigrid coarse levels
replicated below 64^3 with one AllGather at the 32^3 level plus one
indirect-DMA z-slice per iteration, and exchanges a 1-plane p halo per MG
iteration via AllGather + per-core index gather.
"""
import sys
sys.path.insert(0, '/opt/trn_rl_repo')
import numpy as np

from concourse import bacc, bass, tile, bass_utils, mybir

NC = 8
D = 128
ZL = D // NC        # 16 local planes
G = 3               # ghost depth of input tiles
ZX = ZL + 2 * G     # 22-slot global frame
YP = 130

f32 = mybir.dt.float32
f32r = mybir.dt.float32r
i32 = mybir.dt.int32
AF = mybir.ActivationFunctionType
ALU = mybir.AluOpType

_M128 = {}
_n = 0
for _nm in ('xp', 'yp', 'zp', 'dp_c2', 'dp_c2b', 'Ap',
            'xm', 'ym', 'zm', 'dm_c2', 'dm_c2b'):
    _M128[_nm] = _n
    _n += 9
for _nm in ('I1', 'Ic1', 'Ic1b', 'Imdiag', 'Ipdiag', 'Iminvdt'):
    _M128[_nm] = _n
    _n += 1
N_M128 = _n
MINUS_STENCILS = ('x', 'y', 'z', 'd')


# ------------------------------------------------------------------ host math
def _band(w, f, size=D, fold=True):
    B = (w[0] * np.eye(size, k=1) + w[1] * np.eye(size) + w[2] * np.eye(size, k=-1))
    if fold:
        B[0, 0] += f * w[0]
        B[size - 1, size - 1] += f * w[2]
    return B


def _band_set(w3, f, scale=1.0, size=D, fold=True):
    out = np.zeros((9, size, size), np.float32)
    for dz in range(3):
        for dy in range(3):
            out[dz * 3 + dy] = scale * _band(w3[dz, dy], f, size, fold)
    return out


def _res_set(w_res, s_in):
    so = s_in // 2
    out = np.zeros((4, s_in, so), np.float32)
    for dz in range(2):
        for dy in range(2):
            for m in range(so):
                for dx in range(2):
                    out[dz * 2 + dy, 2 * m + dx, m] = w_res[dz, dy, dx]
    return out


def _prol_mat(s):
    P = np.zeros((s, 2 * s), np.float32)
    for k in range(s):
        P[k, 2 * k] = 1.0
        P[k, 2 * k + 1] = 1.0
    return P


def _host_prep(inputs):
    gv = lambda k: np.asarray(inputs[k], np.float32).reshape(D, D, D)
    vu, vv_, vw, vp = gv('values_u'), gv('values_v'), gv('values_w'), gv('values_p')
    sg = gv('sigma')
    w_x = np.asarray(inputs['w_xadv'], np.float64).reshape(3, 3, 3)
    w_y = np.asarray(inputs['w_yadv'], np.float64).reshape(3, 3, 3)
    w_z = np.asarray(inputs['w_zadv'], np.float64).reshape(3, 3, 3)
    w_d = np.asarray(inputs['w_diff'], np.float64).reshape(3, 3, 3)
    wA = np.asarray(inputs['wA'], np.float64).reshape(3, 3, 3)
    w_res = np.asarray(inputs['w_res'], np.float64).reshape(2, 2, 2)
    dt = float(np.asarray(inputs['dt']).reshape(-1)[0])
    iteration = int(inputs['iteration'])
    nlevel = int(inputs['nlevel'])

    S = float(w_d.sum())
    diag = float(wA[1, 1, 1])
    consts = dict(dt=dt, S=S, diag=diag,
                  c1=1.0 - 0.00025 * dt * S, c2=0.0005 * dt,
                  c2b=0.001 * dt, c1b=-0.0005 * dt * S,
                  iteration=iteration, nlevel=nlevel)

    m128 = np.zeros((N_M128, D, D), np.float32)
    m128[_M128['xp']:_M128['xp'] + 9] = _band_set(w_x, 1.0)
    m128[_M128['yp']:_M128['yp'] + 9] = _band_set(w_y, 1.0)
    m128[_M128['zp']:_M128['zp'] + 9] = _band_set(w_z, 1.0)
    m128[_M128['dp_c2']:_M128['dp_c2'] + 9] = _band_set(w_d, 1.0, consts['c2'])
    m128[_M128['dp_c2b']:_M128['dp_c2b'] + 9] = _band_set(w_d, 1.0, consts['c2b'])
    m128[_M128['Ap']:_M128['Ap'] + 9] = _band_set(wA, 1.0)
    m128[_M128['xm']:_M128['xm'] + 9] = _band_set(w_x, -1.0)
    m128[_M128['ym']:_M128['ym'] + 9] = _band_set(w_y, -1.0)
    m128[_M128['zm']:_M128['zm'] + 9] = _band_set(w_z, -1.0)
    m128[_M128['dm_c2']:_M128['dm_c2'] + 9] = _band_set(w_d, -1.0, consts['c2'])
    m128[_M128['dm_c2b']:_M128['dm_c2b'] + 9] = _band_set(w_d, -1.0, consts['c2b'])
    I = np.eye(D, dtype=np.float32)
    m128[_M128['I1']] = I
    m128[_M128['Ic1']] = consts['c1'] * I
    m128[_M128['Ic1b']] = consts['c1b'] * I
    m128[_M128['Imdiag']] = (-1.0 / diag) * I
    m128[_M128['Ipdiag']] = (1.0 / diag) * I
    m128[_M128['Iminvdt']] = (-1.0 / dt) * I

    base = dict(m128=m128,
                res0=_res_set(w_res, 128), res1=_res_set(w_res, 64),
                prol64p=_prol_mat(64).astype(np.float32),
                prol64n=(-_prol_mat(64)).astype(np.float32))
    for s in (32, 16, 8, 4, 2):
        base[f'resc{s}'] = _res_set(w_res, s)
    for s in (64, 32, 16, 8, 4, 2):
        m = np.zeros((11, s, s), np.float32)
        m[:9] = _band_set(wA, 0.0, -1.0 / diag, s, fold=False)
        m[9] = np.eye(s, dtype=np.float32)
        m[10] = np.eye(s, dtype=np.float32) / diag
        base[f'mco{s}'] = m
    for s in (1, 2, 4, 8, 16, 32):
        base[f'prol{s}'] = _prol_mat(s).astype(np.float32)

    ws = {'x': w_x, 'y': w_y, 'z': w_z, 'd': w_d}
    L12 = np.zeros((4, 2, 12, D), np.float32)
    for si, sn in enumerate(MINUS_STENCILS):
        w3 = ws[sn]
        for yc in range(2):
            dyt = 0 if yc == 0 else 2
            for dz in range(3):
                for xe in range(2):
                    m = 0 if xe == 0 else D - 1
                    xet = 0 if xe == 0 else 2
                    L12[si, yc, dz * 4 + yc * 2 + xe, m] = 2.0 * w3[dz, dyt, xet]
    base['L12'] = L12
    eI = np.zeros((D, D), np.float32)
    eI[0, 0] = 1.0
    eI[D - 1, D - 1] = 1.0
    base['edgeI'] = eI


    def shard(gf):
        out = []
        for c in range(NC):
            zmin = c * ZL - G
            idx = np.clip(np.arange(zmin, zmin + ZX), 0, D - 1)
            out.append(np.ascontiguousarray(
                np.transpose(gf[idx], (2, 0, 1)).astype(np.float32)))
        return out

    sh_u, sh_v, sh_w, sh_p, sh_s = (shard(a) for a in (vu, vv_, vw, vp, sg))

    in_maps = []
    for c in range(NC):
        is_bot, is_top = c == 0, c == NC - 1
        vmv = np.zeros(10, np.float32)
        vmv[0] = 0.0 if is_bot else 1.0
        vmv[1] = -1.0 if is_bot else 0.0
        vmv[2] = 1.0 if is_bot else 0.0
        vmv[3] = 0.0 if is_top else 1.0
        vmv[4] = -1.0 if is_top else 0.0
        vmv[5] = 1.0 if is_top else 0.0
        vmv[6] = -1.0 if is_bot else 1.0
        vmv[7] = -1.0 if is_top else 1.0
        vmv[8] = 1.0 if is_bot else -1.0
        vmv[9] = 1.0 if is_top else -1.0

        corrb = np.zeros((D, 24), np.float32)
        for si, sn in enumerate(MINUS_STENCILS):
            w3 = ws[sn]
            for side in range(2):
                mask = 1.0 if (is_bot if side == 0 else is_top) else 0.0
                dzt = 0 if side == 0 else 2
                for dy in range(3):
                    col = si * 6 + side * 3 + dy
                    corrb[0, col] = mask * 2.0 * w3[dzt, dy, 0]
                    corrb[D - 1, col] = mask * 2.0 * w3[dzt, dy, 2]

        cbv = np.zeros((D, 24), np.float32)
        for si, sn in enumerate(MINUS_STENCILS):
            w3 = ws[sn]
            for side in range(2):
                mask = 1.0 if (is_bot if side == 0 else is_top) else 0.0
                dzt = 0 if side == 0 else 2
                for dy in range(3):
                    col = si * 6 + side * 3 + dy
                    cbv[0, col] = mask * 2.0 * w3[dzt, dy, 0]
                    cbv[D - 1, col] = mask * 2.0 * w3[dzt, dy, 2]
        xs = np.arange(D, dtype=np.int32)
        bot_pl = 2 * (c - 1) + 1 if c > 0 else 0
        top_pl = 2 * (c + 1) + 0 if c < NC - 1 else 2 * c + 1
        idx_ph = np.stack([bot_pl * D + xs, top_pl * D + xs], 1).astype(np.int32)

        idx_w = np.zeros((32, 6), np.int32)
        for j, z in enumerate(range(4 * c - 1, 4 * c + 5)):
            idx_w[:, j] = (z * 32 + np.arange(32)) if 0 <= z < 32 else (32 * 32 + np.arange(32))

        m = dict(base)
        m.update(fld_u=sh_u[c], fld_v=sh_v[c], fld_w=sh_w[c], fld_p=sh_p[c],
                 fld_s=sh_s[c], vm=vmv, cb=cbv, idx_ph=idx_ph, idx_w=idx_w)
        in_maps.append(m)
    return in_maps, consts


# ------------------------------------------------------------------ builder
def _build(consts):
    global DEBUG
    dt = consts['dt']
    diag = consts['diag']
    iteration = consts['iteration']

    nc = bacc.Bacc("TRN2", target_bir_lowering=False, debug=False, num_devices=NC)
    dr = {}

    def din(name, shape, dtp=f32r):
        dr[name] = nc.dram_tensor(name, list(shape), dtp, kind="ExternalInput")

    for nm in ('fld_u', 'fld_v', 'fld_w', 'fld_p'):
        din(nm, (D, ZX, D))
    din('fld_s', (D, ZX, D), f32)
    din('m128', (N_M128, D, D))
    din('res0', (4, 128, 64), f32); din('res1', (4, 64, 32), f32)
    for s in (32, 16, 8, 4, 2):
        din(f'resc{s}', (4, s, s // 2), f32)
    for s in (64, 32, 16, 8, 4, 2):
        din(f'mco{s}', (11, s, s))
    for s in (1, 2, 4, 8, 16, 32):
        din(f'prol{s}', (s, 2 * s), f32)
    din('prol64p', (64, 128), f32); din('prol64n', (64, 128), f32)
    din('L12', (4, 2, 12, D), f32)
    din('cb', (D, 24), f32)
    din('edgeI', (D, D))
    din('vm', (10,), f32)
    din('idx_ph', (D, 2), i32); din('idx_w', (32, 6), i32)

    out_f = nc.dram_tensor("out_fields", [5, D, ZL, D], f32, kind="ExternalOutput")
    out_r = nc.dram_tensor("out_r", [1, 1], f32, kind="ExternalOutput")
    dbg = {}
    if DEBUG:
        for nm, shp in DEBUG.items():
            dbg[nm] = nc.dram_tensor("dbg_" + nm, list(shp), f32,
                                     kind="ExternalOutput")

    DMAE = [nc.sync, nc.scalar, nc.gpsimd]
    dma_i = [0]

    def dma(dst, src):
        DMAE[dma_i[0] % len(DMAE)].dma_start(dst, src)
        dma_i[0] += 1

    with tile.TileContext(nc) as tc:
        with (
            tc.tile_pool(name="pool", bufs=1) as P_,
            tc.tile_pool(name="mats", bufs=2) as MP,
            tc.tile_pool(name="tmp", bufs=2) as TP,
            tc.tile_pool(name="psum", bufs=4, space="PSUM") as PS,
            tc.tile_pool(name="psc", bufs=2, space="PSUM") as PSC,
            tc.tile_pool(name="dram", bufs=1, space="DRAM") as DP,
        ):
            r32 = lambda ap: ap.bitcast(f32)

            ids = P_.tile([D, 6, D], f32r, tag="ids")
            dma(ids[:], dr['m128'][_M128['I1']:_M128['I1'] + 6]
                .rearrange("i k m -> k i m"))

            def ID(nm):
                return ids[:, _M128[nm] - _M128['I1'], :]

            rs0 = P_.tile([128, 4, 64], f32, tag="rs0")
            dma(rs0[:], dr['res0'][:].rearrange("i k m -> k i m"))
            rs1 = P_.tile([64, 4, 32], f32, tag="rs1")
            dma(rs1[:], dr['res1'][:].rearrange("i k m -> k i m"))
            rsc, mco, prl = {}, {}, {}
            for s in (32, 16, 8, 4, 2):
                rsc[s] = P_.tile([s, 4, s // 2], f32, tag=f"rsc{s}", name=f"rsc{s}")
                dma(rsc[s][:], dr[f'resc{s}'][:].rearrange("i k m -> k i m"))
            for s in (64, 32, 16, 8, 4, 2):
                mco[s] = P_.tile([s, 11, s], f32r, tag=f"mco{s}", name=f"mco{s}")
                dma(mco[s][:], dr[f'mco{s}'][:].rearrange("i k m -> k i m"))
            for s in (1, 2, 4, 8, 16, 32):
                prl[s] = P_.tile([s, 2 * s], f32, tag=f"prl{s}", name=f"prl{s}")
                dma(prl[s][:], dr[f'prol{s}'][:])
            p64p = P_.tile([64, 128], f32, tag="p64p")
            dma(p64p[:], dr['prol64p'][:])
            p64n = P_.tile([64, 128], f32, tag="p64n")
            dma(p64n[:], dr['prol64n'][:])
            L12 = P_.tile([12, 8, D], f32, tag="L12")
            dma(L12[:], dr['L12'][:].rearrange("s c k m -> k (s c) m"))

            cb = P_.tile([D, 24], f32, tag="cb")
            dma(cb[:], dr['cb'][:])
            edgeI = P_.tile([D, D], f32r, tag="edgeI")
            dma(edgeI[:], dr['edgeI'][:])
            vm = P_.tile([D, 10], f32, tag="vm")
            dma(vm[:], dr['vm'][:].partition_broadcast(D))
            ixp = P_.tile([D, 2], i32, tag="ixp")
            dma(ixp[:], dr['idx_ph'][:])
            ixw = P_.tile([32, 6], i32, tag="ixw")
            dma(ixw[:], dr['idx_w'][:])


            class FT:
                def __init__(self, ap, z0):
                    self.ap = ap
                    self.z0 = z0
                    self.shape = ap.shape

                def __getitem__(self, k):
                    return self.ap[k]

            def ftile(tag, z0, zn):
                t = P_.tile([D, zn, YP], f32r, tag=tag, name=tag)
                return FT(t, z0)

            U = {n: ftile('t' + n, 0, ZX) for n in 'uvw'}
            p = ftile('p', 0, ZX)
            rd = P_.tile([D, ZX, D], f32, tag="rd")
            B_ = {n: ftile('b' + n, 1, 20) for n in 'uvw'}
            U2 = {n: ftile('q' + n, 2, 18) for n in 'uvw'}
            xps = {}
            for n in 'uvw':
                t = P_.tile([D, 20, D], f32, tag='xp' + n,
                            name='xp' + n)
                xps[n] = FT(t, 1)

            for n, src in (('u', 'fld_u'), ('v', 'fld_v'), ('w', 'fld_w')):
                dma(U[n][:, :, 1:129], dr[src][:])
            dma(p[:, :, 1:129], dr['fld_p'][:])
            dma(rd[:], dr['fld_s'][:])
            nc.vector.tensor_scalar(rd[:], rd[:], float(dt), 1.0,
                                    op0=ALU.mult, op1=ALU.add)
            nc.vector.reciprocal(rd[:], rd[:])
            for n in 'uvw':
                nc.vector.tensor_mul(U[n][:, :, 1:129], r32(U[n][:, :, 1:129]), rd[:])
            for (sl, vi) in ((slice(0, G), 6), (slice(ZX - G, ZX), 7)):
                nc.scalar.activation(U['u'][:, sl, 1:129], r32(U['u'][:, sl, 1:129]),
                                     AF.Copy, scale=vm[:, vi:vi + 1])

            def ycols(t, a, b_, f, gslots=()):
                la, lb = a - t.z0, b_ - t.z0
                nc.scalar.activation(t[:, la:lb, 0:1], r32(t[:, la:lb, 1:2]),
                                     AF.Copy, scale=float(f))
                nc.scalar.activation(t[:, la:lb, 129:130], r32(t[:, la:lb, 128:129]),
                                     AF.Copy, scale=float(f))
                for (g, vi) in gslots:
                    lg = g - t.z0
                    nc.scalar.activation(t[:, lg:lg + 1, 0:1],
                                         r32(t[:, lg:lg + 1, 1:2]),
                                         AF.Copy, scale=vm[:, vi:vi + 1])
                    nc.scalar.activation(t[:, lg:lg + 1, 129:130],
                                         r32(t[:, lg:lg + 1, 128:129]),
                                         AF.Copy, scale=vm[:, vi:vi + 1])

            ycols(U['u'], G, ZX - G, -1.0,
                  gslots=[(g, 8) for g in range(G)]
                  + [(g, 9) for g in range(ZX - G, ZX)])
            ycols(U['v'], 0, ZX, 1.0)
            ycols(U['w'], 0, ZX, 1.0)
            ycols(p, 0, ZX, 1.0)

            def cc3(t, o0, zout):
                c = TP.tile([12, zout], f32r, tag="cc3", bufs=1)
                for dz in range(3):
                    for yc in range(2):
                        for xe in range(2):
                            k = dz * 4 + yc * 2 + xe
                            m = 0 if xe == 0 else D - 1
                            ycol = 0 if yc == 0 else 129
                            ls = o0 - 1 + dz - t.z0
                            nc.sync.dma_start(
                                c[k:k + 1, 0:zout],
                                t[m:m + 1, ls:ls + zout, ycol:ycol + 1]
                                .rearrange("p z o -> p (z o)"))
                return c

            def mset(nm):
                t = MP.tile([D, 9, D], f32r, tag="mset")
                b0 = _M128[nm]
                dma(t[:], dr['m128'][b0:b0 + 9].rearrange("i k m -> k i m"))
                return t

            def conv_groups(T, o0, zout, mt, id_terms=(), corr=None):
                for a in range(0, zout, 4):
                    cp = min(4, zout - a)
                    ps = PS.tile([D, 4, D], f32, tag="cv")
                    ops = []
                    for dz in range(3):
                        for dy in range(3):
                            ls = o0 + a - 1 + dz - T.z0
                            ops.append((mt[:, dz * 3 + dy, :],
                                        T[:, ls:ls + cp, dy:dy + D],
                                        ps[:, 0:cp, :]))
                    for (iap, src) in id_terms:
                        lo = o0 + a - src.z0
                        ops.append((iap, src[:, lo:lo + cp, 1:129], ps[:, 0:cp, :]))
                    if corr is not None:
                        si, cct, srcT = corr
                        for yc in range(2):
                            yo = 0 if yc == 0 else D - 1
                            ops.append((L12[:, si * 2 + yc, :],
                                        cct[:, a:a + cp].bitcast(f32),
                                        ps[:, 0:cp, yo:yo + 1]))
                    n = len(ops)
                    for i, (lh, rh, o_ap) in enumerate(ops):
                        nc.tensor.matmul(o_ap, lh, rh,
                                         start=(i == 0), stop=(i == n - 1))
                    yield a, o0 + a, cp, ps

            def full_conv(T, o0, zout, mt, dst_fn, scale=1.0):
                for a, og, cp, ps in conv_groups(T, o0, zout, mt):
                    nc.scalar.activation(dst_fn(a, cp), ps[:, 0:cp, :],
                                         AF.Copy, scale=float(scale))

            # ---------------- stage 1 + 2 ----------------
            def stage_combine(comp, T, o0, zout, PROD, msets, mdf, id_terms,
                              dtc, xpsrc, dst, cc):
                is_minus = comp == 'u'
                mtile = TP.tile([D, zout, D], f32, tag="madv", bufs=1)
                for k, (sn, mul) in enumerate((('x', 'u'), ('y', 'v'), ('z', 'w'))):
                    corr = (MINUS_STENCILS.index(sn), cc, T) if is_minus else None
                    mt_k = mset(msets[sn])
                    for a, og, cp, ps in conv_groups(T, o0, zout, mt_k,
                                                     corr=corr):
                        mu = PROD[mul]
                        msl = r32(mu[:, og - mu.z0: og - mu.z0 + cp, 1:129])
                        if k == 0:
                            nc.vector.tensor_mul(mtile[:, a:a + cp, :], msl,
                                                 ps[:, 0:cp, :])
                        else:
                            tt = TP.tile([D, 4, D], f32, tag="advt", bufs=1)
                            nc.vector.tensor_mul(tt[:, 0:cp, :], msl, ps[:, 0:cp, :])
                            nc.gpsimd.tensor_add(mtile[:, a:a + cp, :],
                                                 mtile[:, a:a + cp, :], tt[:, 0:cp, :])
                corr = (3, cc, T) if is_minus else None
                mt_d = mset(mdf)
                for a, og, cp, ps in conv_groups(T, o0, zout, mt_d,
                                                 id_terms=id_terms, corr=corr):
                    nc.vector.scalar_tensor_tensor(
                        mtile[:, a:a + cp, :], mtile[:, a:a + cp, :], float(-dtc),
                        ps[:, 0:cp, :], op0=ALU.mult, op1=ALU.add)
                nc.gpsimd.tensor_add(mtile[:], mtile[:],
                                     xpsrc[:, o0 - xpsrc.z0:o0 - xpsrc.z0 + zout, :])
                nc.vector.tensor_mul(dst[:, o0 - dst.z0:o0 - dst.z0 + zout, 1:129],
                                     mtile[:], rd[:, o0:o0 + zout, :])

            for sn, n in (('xp', 'u'), ('yp', 'v'), ('zp', 'w')):
                mt = mset(sn)
                xx = xps[n]
                full_conv(p, 1, 20, mt,
                          lambda a, cp, _x=xx: _x[:, a:a + cp, :], scale=-dt)

            cc_u = cc3(U['u'], 1, 20)
            for comp in 'uvw':
                ms = ({'x': 'xm', 'y': 'ym', 'z': 'zm'} if comp == 'u'
                      else {'x': 'xp', 'y': 'yp', 'z': 'zp'})
                stage_combine(comp, U[comp], 1, 20, U, ms,
                              'dm_c2' if comp == 'u' else 'dp_c2',
                              [(ID('Ic1'), U[comp])],
                              0.5 * dt, xps[comp], B_[comp],
                              cc_u if comp == 'u' else None)

            def blend(t, slots_bot, slots_top, minus):
                eb = t[:, G - t.z0:G - t.z0 + 1, 1:129]
                et = t[:, G + ZL - 1 - t.z0:G + ZL - t.z0, 1:129]
                for (slots, edge, mvi, svi) in (
                        (slots_bot, eb, 0, 1 if minus else 2),
                        (slots_top, et, 3, 4 if minus else 5)):
                    for g in slots:
                        sl = t[:, g - t.z0:g - t.z0 + 1, 1:129]
                        nc.scalar.activation(sl, r32(sl), AF.Copy,
                                             scale=vm[:, mvi:mvi + 1])
                        nc.vector.scalar_tensor_tensor(
                            sl, r32(edge), vm[:, svi:svi + 1], r32(sl),
                            op0=ALU.mult, op1=ALU.add)

            if DEBUG:
                nc.sync.dma_start(dbg['bu'][:], r32(B_['u'][:, :, 1:129]))
                xau = TP.tile([D, 20, D], f32, tag="madv", bufs=1)
                mtl = mset('xm')
                cc_d = cc3(U['u'], 1, 20)
                for a, og, cp, ps in conv_groups(U['u'], 1, 20, mtl,
                                                 corr=(0, cc_d, U['u'])):
                    nc.scalar.activation(xau[:, a:a + cp, :], ps[:, 0:cp, :],
                                         AF.Copy)
                nc.sync.dma_start(dbg['xau'][:], xau[:])
                nc.sync.dma_start(dbg['xpu'][:], xps['u'][:])
            blend(B_['u'], (1, 2), (19, 20), True)
            blend(B_['v'], (1, 2), (19, 20), False)
            blend(B_['w'], (1, 2), (19, 20), False)
            ycols(B_['u'], G, ZX - G, -1.0,
                  gslots=[(1, 8), (2, 8), (19, 9), (20, 9)])
            ycols(B_['v'], 1, 21, 1.0)
            ycols(B_['w'], 1, 21, 1.0)

            cc_b = cc3(B_['u'], 2, 18)
            for comp in 'uvw':
                ms = ({'x': 'xm', 'y': 'ym', 'z': 'zm'} if comp == 'u'
                      else {'x': 'xp', 'y': 'yp', 'z': 'zp'})
                stage_combine(comp, B_[comp], 2, 18, B_, ms,
                              'dm_c2b' if comp == 'u' else 'dp_c2b',
                              [(ID('Ic1b'), B_[comp]), (ID('I1'), U[comp])],
                              dt, xps[comp], U2[comp],
                              cc_b if comp == 'u' else None)

            if DEBUG:
                nc.sync.dma_start(dbg['qu'][:], r32(U2['u'][:, :, 1:129]))
            blend(U2['u'], (2,), (19,), True)
            blend(U2['v'], (2,), (19,), False)
            blend(U2['w'], (2,), (19,), False)
            ycols(U2['u'], G, ZX - G, -1.0, gslots=[(2, 8), (19, 9)])
            ycols(U2['v'], 2, 20, 1.0)
            ycols(U2['w'], 2, 20, 1.0)

            # ---------------- stage 3: b ----------------
            b = FT(P_.tile([D, ZL, D], f32r, tag="bu", name="stb"), G)
            mt_x3, mt_y3, mt_z3 = mset('xm'), mset('yp'), mset('zp')
            cc_q = cc3(U2['u'], G, ZL)
            for a in range(0, ZL, 4):
                og = G + a
                ps = PS.tile([D, 4, D], f32, tag="cv")
                ops = []
                for (mt, T) in ((mt_x3, U2['u']), (mt_y3, U2['v']), (mt_z3, U2['w'])):
                    for dz in range(3):
                        for dy in range(3):
                            ls = og - 1 + dz - T.z0
                            ops.append((mt[:, dz * 3 + dy, :],
                                        T[:, ls:ls + 4, dy:dy + D], ps[:]))
                for yc in range(2):
                    yo = 0 if yc == 0 else D - 1
                    ops.append((L12[:, 0 * 2 + yc, :],
                                cc_q[:, a:a + 4].bitcast(f32),
                                ps[:, :, yo:yo + 1]))
                n = len(ops)
                for i, (lh, rh, o_ap) in enumerate(ops):
                    nc.tensor.matmul(o_ap, lh, rh, start=(i == 0), stop=(i == n - 1))
                nc.scalar.activation(b[:, a:a + 4, :], ps[:], AF.Copy,
                                     scale=float(-1.0 / dt))
                for side, ogx in ((0, G), (1, G + ZL - 1)):
                    pos = ogx - og
                    if 0 <= pos < 4:
                        gs = (G - 1 if side == 0 else ZX - G) - U2['u'].z0
                        psb = PS.tile([D, 1, D], f32, tag="cvb", name="psb",
                                      bufs=2)
                        for dy in range(3):
                            col = side * 3 + dy
                            ct = TP.tile([D, 1, D], f32r, tag="cbt", bufs=3,
                                         name="cbt")
                            nc.scalar.activation(
                                ct[:], r32(U2['u'][:, gs:gs + 1, dy:dy + D]),
                                AF.Copy, scale=cb[:, col:col + 1])
                            nc.tensor.matmul(psb[:], edgeI[:], ct[:],
                                             start=(dy == 0), stop=(dy == 2))
                        nc.vector.scalar_tensor_tensor(
                            b[:, a + pos:a + pos + 1, :], psb[:],
                            float(-1.0 / dt), r32(b[:, a + pos:a + pos + 1, :]),
                            op0=ALU.mult, op1=ALU.add)

            # ---------------- multigrid ----------------
            App = FT(P_.tile([D, ZL, D], f32r, tag="bv", name="app"), G)
            if DEBUG:
                nc.sync.dma_start(dbg['b'][:], r32(b[:]))
            wmg = P_.tile([D, ZL, D], f32, tag="bw")


            wpad = {}
            for s in (4, 8, 16, 32):
                wpad[s] = P_.tile([s, s + 2, s + 2], f32r, tag=f"wp{s}", name=f"wp{s}")
                nc.vector.memset(wpad[s][:].bitcast(f32), 0.0)
                nc.vector.tensor_copy(wpad[s][:], wpad[s][:].bitcast(f32))
            wpad[2] = P_.tile([2, 4, 4], f32r, tag="wp2", name="wp2")
            nc.vector.memset(wpad[2][:].bitcast(f32), 0.0)
            nc.vector.tensor_copy(wpad[2][:], wpad[2][:].bitcast(f32))
            w64pad = P_.tile([64, 12, 66], f32r, tag="w64p")
            nc.vector.memset(w64pad[:].bitcast(f32), 0.0)
            nc.vector.tensor_copy(w64pad[:], w64pad[:].bitcast(f32))
            w32sl = P_.tile([32, 6, 32], f32r, tag="w32sl")
            zrow32 = P_.tile([32, 32], f32, tag="zrow32")
            nc.vector.memset(zrow32[:], 0.0)
            r7_out = P_.tile([1, 1], f32, tag="r7o")

            def prol_mm(lh, src, zsl, parts, zn, width, ps, first, last):
                """prol: out[2z+a, 2y+b] = src[z, y]; 2 matmuls (a=0,1)."""
                pv = ps[:].rearrange("m (z a) y -> m a z y", a=2)
                rh = (src[:, zsl, :].bitcast(f32).unsqueeze(3)
                      .broadcast_to([parts, zn, width, 2]))
                for a in range(2):
                    nc.tensor.matmul(pv[:, a], lh, rh,
                                     start=(first and a == 0),
                                     stop=(last and a == 1))

            for it in range(iteration):
                ph_in = DP.tile([2 * D * D], f32, tag=f"phin{it}", name=f"phin{it}")
                ph_out = DP.tile([NC, 2 * D * D], f32, tag=f"phout{it}",
                                 name=f"phout{it}", addr_space="Shared")
                r2_in = DP.tile([4 * 32 * 32], f32, tag=f"r2in{it}",
                                name=f"r2in{it}")
                r2_out = DP.tile([NC, 4 * 32 * 32], f32, tag=f"r2out{it}",
                                 name=f"r2out{it}", addr_space="Shared")
                w32_d = DP.tile([33 * 32, 32], f32, tag=f"w32d{it}",
                                name=f"w32d{it}")
                apm = mset('Ap')
                full_conv(p, G, ZL, apm,
                          lambda a, cp: App[:, a:a + cp, :])
                rzero = P_.tile([D, ZL, D], f32r, tag="tv")
                nc.vector.tensor_sub(rzero[:], r32(App[:]), r32(b[:]))
                r1 = TP.tile([64, 8, 64], f32r, tag="r1", bufs=1)
                ps = PSC.tile([64, 8, 64], f32, tag="co")
                rzv = r32(rzero[:]).rearrange("k (z a) (y c) -> k a c z y", a=2, c=2)
                for i in range(4):
                    nc.tensor.matmul(ps[:], rs0[:, i, :],
                                     rzv[:, i // 2, i % 2],
                                     start=(i == 0), stop=(i == 3))
                nc.scalar.activation(r1[:], ps[:], AF.Copy)
                r2 = TP.tile([32, 4, 32], f32r, tag="r2", bufs=1)
                ps = PSC.tile([32, 4, 32], f32, tag="co")
                r1v = r32(r1[:]).rearrange("k (z a) (y c) -> k a c z y", a=2, c=2)
                for i in range(4):
                    nc.tensor.matmul(ps[:], rs1[:, i, :],
                                     r1v[:, i // 2, i % 2],
                                     start=(i == 0), stop=(i == 3))
                nc.scalar.activation(r2[:], ps[:], AF.Copy)

                nc.sync.dma_start(
                    r2_in[:].rearrange("(z x y) -> x z y", z=4, x=32, y=32),
                    r32(r2[:]))
                nc.gpsimd.collective_compute(
                    "AllGather", ALU.bypass, replica_groups=[list(range(NC))],
                    ins=[r2_in[:].opt()], outs=[r2_out[:].opt()])
                r2f = P_.tile([32, 32, 32], f32r, tag="tw", name="r2f")
                nc.sync.dma_start(
                    r2f[:],
                    r2_out[:].rearrange("c (z x y) -> x (c z) y", z=4, x=32, y=32)
                    .bitcast(f32r))

                rl = {32: r2f}
                src = r2f
                for s in (32, 16, 8, 4, 2):
                    so = s // 2
                    dstr = TP.tile([so, so, so], f32r, tag=f"rv{so}", name=f"rv{so}")
                    ps = PSC.tile([so, so, so], f32, tag="co")
                    sv = r32(src[:]).rearrange("k (z a) (y c) -> k a c z y",
                                               a=2, c=2)
                    for i in range(4):
                        nc.tensor.matmul(ps[:], rsc[s][:, i, :],
                                         sv[:, i // 2, i % 2],
                                         start=(i == 0), stop=(i == 3))
                    nc.scalar.activation(dstr[:], ps[:], AF.Copy)
                    rl[so] = dstr
                    src = dstr
                if it == iteration - 1:
                    nc.scalar.activation(r7_out[:].unsqueeze(1), r32(rl[1][:]),
                                         AF.Copy)

                wcur = TP.tile([1, 1, 1], f32r, tag="w1")
                nc.scalar.activation(wcur[:], r32(rl[1][:]), AF.Copy,
                                     scale=float(1.0 / diag))
                for s in (1, 2, 4, 8, 16):
                    s2 = 2 * s
                    nhalf = 2 if s2 == 32 else 1
                    for h in range(nhalf):
                        zh = s2 // nhalf
                        ps = PSC.tile([s2, zh, s2], f32, tag="co")
                        prol_mm(prl[s][:], wcur,
                                slice(h * zh // 2, (h + 1) * zh // 2),
                                s, zh // 2, s, ps, True, True)
                        nc.scalar.activation(
                            wpad[s2][:, 1 + h * zh:1 + (h + 1) * zh, 1:s2 + 1],
                            ps[:], AF.Copy)
                    wn = TP.tile([32, 32, 32], f32r, tag="wu", name=f"wu{s2}", bufs=1)[0:s2, 0:s2, 0:s2]
                    for h in range(nhalf):
                        zh = s2 // nhalf
                        ps2 = PSC.tile([s2, zh, s2], f32, tag="co")
                        k = 0
                        for dz in range(3):
                            for dy in range(3):
                                nc.tensor.matmul(
                                    ps2[:], mco[s2][:, dz * 3 + dy, :],
                                    wpad[s2][:, h * zh + dz:h * zh + dz + zh,
                                             dy:dy + s2],
                                    start=(k == 0), stop=False)
                                k += 1
                        nc.tensor.matmul(ps2[:], mco[s2][:, 9, :],
                                         wpad[s2][:, 1 + h * zh:1 + (h + 1) * zh,
                                                  1:s2 + 1],
                                         start=False, stop=False)
                        nc.tensor.matmul(ps2[:], mco[s2][:, 10, :],
                                         rl[s2][:, h * zh:(h + 1) * zh, :],
                                         start=False, stop=True)
                        nc.scalar.activation(wn[:, h * zh:(h + 1) * zh, :], ps2[:],
                                             AF.Copy)
                    wcur = wn

                nc.sync.dma_start(
                    w32_d[0:1024].rearrange("(z x) y -> x z y", z=32, x=32),
                    r32(wcur[:]))
                nc.sync.dma_start(w32_d[1024:1056], zrow32[:])
                nc.vector.memset(w32sl[:].bitcast(f32), 0.0)
                nc.vector.tensor_copy(w32sl[:], w32sl[:].bitcast(f32))
                for j in range(6):
                    gt = TP.tile([D, D], f32, tag="gat", name="gt32", bufs=1)[0:32, 0:32]
                    nc.gpsimd.indirect_dma_start(
                        out=gt[:], out_offset=None, in_=w32_d[:],
                        in_offset=bass.IndirectOffsetOnAxis(ap=ixw[:, j:j + 1],
                                                            axis=0))
                    nc.vector.tensor_copy(w32sl[:, j:j + 1, :], gt[:].unsqueeze(1))
                for h in range(2):
                    ps = PSC.tile([64, 6, 64], f32, tag="co")
                    prol_mm(prl[32][:], w32sl, slice(3 * h, 3 * h + 3),
                            32, 3, 32, ps, True, True)
                    nc.scalar.activation(w64pad[:, 6 * h:6 * h + 6, 1:65], ps[:],
                                         AF.Copy)
                ps2 = PSC.tile([64, 8, 64], f32, tag="co")
                k = 0
                for dz in range(3):
                    for dy in range(3):
                        nc.tensor.matmul(ps2[:], mco[64][:, dz * 3 + dy, :],
                                         w64pad[:, 1 + dz:1 + dz + 8, dy:dy + 64],
                                         start=(k == 0), stop=False)
                        k += 1
                nc.tensor.matmul(ps2[:], mco[64][:, 9, :], w64pad[:, 2:10, 1:65],
                                 start=False, stop=False)
                nc.tensor.matmul(ps2[:], mco[64][:, 10, :], r1[:],
                                 start=False, stop=True)
                w64u = TP.tile([64, 8, 64], f32r, tag="w64u", bufs=1)
                nc.scalar.activation(w64u[:], ps2[:], AF.Copy)
                if DEBUG and it == 0:
                    nc.sync.dma_start(dbg['App'][:], r32(App[:]))
                    nc.sync.dma_start(dbg['r2f'][:], r32(r2f[:]))
                    nc.sync.dma_start(dbg['w32'][:], r32(wcur[:]))
                    nc.sync.dma_start(dbg['w64u'][:], r32(w64u[:]))

                last = it == iteration - 1
                for a in range(0, ZL, 4):
                    og = G + a
                    if last:
                        psw = PS.tile([D, 4, D], f32, tag="cv")
                        prol_mm(p64p[:], w64u, slice(a // 2, a // 2 + 2),
                                64, 2, 64, psw, True, True)
                        nc.scalar.activation(wmg[:, a:a + 4, :], psw[:], AF.Copy)
                    psp = PS.tile([D, 4, D], f32, tag="cv")
                    nc.tensor.matmul(psp[:], ID('I1'), p[:, og:og + 4, 1:129],
                                     start=True, stop=False)
                    nc.tensor.matmul(psp[:], ID('Imdiag'), App[:, a:a + 4, :],
                                     start=False, stop=False)
                    nc.tensor.matmul(psp[:], ID('Ipdiag'), b[:, a:a + 4, :],
                                     start=False, stop=False)
                    prol_mm(p64n[:], w64u, slice(a // 2, a // 2 + 2),
                            64, 2, 64, psp, False, True)
                    nc.scalar.activation(p[:, og:og + 4, 1:129], psp[:], AF.Copy)

                if DEBUG and it == 0:
                    nc.sync.dma_start(dbg['p0'][:], r32(p[:, G:G + ZL, 1:129]))
                nc.sync.dma_start(
                    ph_in[0:D * D].rearrange("(x y) -> x y", x=D),
                    r32(p[:, G:G + 1, 1:129]).rearrange("x o y -> x (o y)"))
                nc.sync.dma_start(
                    ph_in[D * D:2 * D * D].rearrange("(x y) -> x y", x=D),
                    r32(p[:, G + ZL - 1:G + ZL, 1:129])
                    .rearrange("x o y -> x (o y)"))
                nc.gpsimd.collective_compute(
                    "AllGather", ALU.bypass, replica_groups=[list(range(NC))],
                    ins=[ph_in[:].opt()], outs=[ph_out[:].opt()])
                phv = ph_out[:].rearrange("c (p x y) -> (c p x) y", p=2, x=D, y=D)
                for (col, slot) in ((0, G - 1), (1, G + ZL)):
                    gt = TP.tile([D, D], f32, tag="gat", bufs=1)
                    nc.gpsimd.indirect_dma_start(
                        out=gt[:], out_offset=None, in_=phv,
                        in_offset=bass.IndirectOffsetOnAxis(
                            ap=ixp[:, col:col + 1], axis=0))
                    nc.vector.tensor_copy(p[:, slot:slot + 1, 1:129],
                                          gt[:].unsqueeze(1))
                ycols(p, G - 1, G + ZL + 1, 1.0)

            # ---------------- final correction ----------------
            for fi, comp in enumerate('uvw'):
                u3 = P_.tile([D, ZL, D], f32, tag="tu", name="u3")
                mt = mset({'u': 'xp', 'v': 'yp', 'w': 'zp'}[comp])
                for a in range(0, ZL, 4):
                    og = G + a
                    ps = PS.tile([D, 4, D], f32, tag="cv")
                    k = 0
                    for dz in range(3):
                        for dy in range(3):
                            nc.tensor.matmul(
                                ps[:], mt[:, dz * 3 + dy, :],
                                p[:, og - 1 + dz:og - 1 + dz + 4, dy:dy + D],
                                start=(k == 0), stop=False)
                            k += 1
                    qq = U2[comp]
                    nc.tensor.matmul(ps[:], ID('Iminvdt'),
                                     qq[:, og - qq.z0:og - qq.z0 + 4, 1:129],
                                     start=False, stop=True)
                    nc.vector.scalar_tensor_tensor(
                        u3[:, a:a + 4, :], ps[:], float(-dt),
                        rd[:, og:og + 4, :], op0=ALU.mult, op1=ALU.mult)
                dma(out_f[fi], u3[:])

            dma(out_f[3], r32(p[:, G:G + ZL, 1:129]))
            dma(out_f[4], wmg[:])
            nc.sync.dma_start(out_r[:], r7_out[:])

    nc.compile()
    return nc


# ------------------------------------------------------------------ entry
_CACHE = {}
DEBUG = {}


def kernel(**inputs):
    in_maps, consts = _host_prep(inputs)
    key = (consts['dt'], consts['S'], consts['diag'], consts['iteration'],
           consts['nlevel'],
           np.asarray(inputs['wA'], np.float32).tobytes(),
           np.asarray(inputs['w_res'], np.float32).tobytes())
    if key not in _CACHE:
        _CACHE[key] = _build(consts)
    res = bass_utils.run_bass_kernel_spmd(_CACHE[key], in_maps,
                                          core_ids=list(range(NC)))

    def unshard(i):
        return np.concatenate(
            [np.transpose(res.results[c]["out_fields"][i], (1, 2, 0))
             for c in range(NC)], axis=0)[None, None]

    u, v, w, pfin, wmg = (unshard(i) for i in range(5))
    r = res.results[0]["out_r"].reshape(1, 1, 1, 1, 1).astype(np.float32)
    return (u.astype(np.float32), v.astype(np.float32), w.astype(np.float32),
            pfin.astype(np.float32), wmg.astype(np.float32), r)


# revision 35
# speedup vs baseline: 1.1710x; 1.1710x over previous
"""Trainium2 8-core Bass kernel for nn_AI4Urban (CFD step + multigrid).

Self-contained: builds per-call (weights/dt baked as compile-time consts),
shards the 128^3 grid along z across 8 NeuronCores with 3-deep ghost input
planes, runs all 3x3x3 stencils as banded f32r matmuls on the PE
(x in partitions, (z,y) in the free dim), does the multigrid coarse levels
replicated below 64^3 with one AllGather at the 32^3 level plus one
indirect-DMA z-slice per iteration, and exchanges a 1-plane p halo per MG
iteration via AllGather + per-core index gather.
"""
import sys
sys.path.insert(0, '/opt/trn_rl_repo')
import numpy as np

from concourse import bacc, bass, tile, bass_utils, mybir

NC = 8
D = 128
ZL = D // NC        # 16 local planes
G = 3               # ghost depth of input tiles
ZX = ZL + 2 * G     # 22-slot global frame
YP = 130

f32 = mybir.dt.float32
f32r = mybir.dt.float32r
i32 = mybir.dt.int32
AF = mybir.ActivationFunctionType
ALU = mybir.AluOpType

_M128 = {}
_n = 0
for _nm in ('xp', 'yp', 'zp', 'dp_c2', 'dp_c2b', 'Ap',
            'xm', 'ym', 'zm', 'dm_c2', 'dm_c2b'):
    _M128[_nm] = _n
    _n += 9
for _nm in ('I1', 'Ic1', 'Ic1b', 'Imdiag', 'Ipdiag', 'Iminvdt'):
    _M128[_nm] = _n
    _n += 1
N_M128 = _n
MINUS_STENCILS = ('x', 'y', 'z', 'd')


# ------------------------------------------------------------------ host math
def _band(w, f, size=D, fold=True):
    B = (w[0] * np.eye(size, k=1) + w[1] * np.eye(size) + w[2] * np.eye(size, k=-1))
    if fold:
        B[0, 0] += f * w[0]
        B[size - 1, size - 1] += f * w[2]
    return B


def _band_set(w3, f, scale=1.0, size=D, fold=True):
    out = np.zeros((9, size, size), np.float32)
    for dz in range(3):
        for dy in range(3):
            out[dz * 3 + dy] = scale * _band(w3[dz, dy], f, size, fold)
    return out


def _res_set(w_res, s_in):
    so = s_in // 2
    out = np.zeros((4, s_in, so), np.float32)
    for dz in range(2):
        for dy in range(2):
            for m in range(so):
                for dx in range(2):
                    out[dz * 2 + dy, 2 * m + dx, m] = w_res[dz, dy, dx]
    return out


def _prol_mat(s):
    P = np.zeros((s, 2 * s), np.float32)
    for k in range(s):
        P[k, 2 * k] = 1.0
        P[k, 2 * k + 1] = 1.0
    return P


def _host_prep(inputs):
    gv = lambda k: np.asarray(inputs[k], np.float32).reshape(D, D, D)
    vu, vv_, vw, vp = gv('values_u'), gv('values_v'), gv('values_w'), gv('values_p')
    sg = gv('sigma')
    w_x = np.asarray(inputs['w_xadv'], np.float64).reshape(3, 3, 3)
    w_y = np.asarray(inputs['w_yadv'], np.float64).reshape(3, 3, 3)
    w_z = np.asarray(inputs['w_zadv'], np.float64).reshape(3, 3, 3)
    w_d = np.asarray(inputs['w_diff'], np.float64).reshape(3, 3, 3)
    wA = np.asarray(inputs['wA'], np.float64).reshape(3, 3, 3)
    w_res = np.asarray(inputs['w_res'], np.float64).reshape(2, 2, 2)
    dt = float(np.asarray(inputs['dt']).reshape(-1)[0])
    iteration = int(inputs['iteration'])
    nlevel = int(inputs['nlevel'])

    S = float(w_d.sum())
    diag = float(wA[1, 1, 1])
    consts = dict(dt=dt, S=S, diag=diag,
                  c1=1.0 - 0.00025 * dt * S, c2=0.0005 * dt,
                  c2b=0.001 * dt, c1b=-0.0005 * dt * S,
                  iteration=iteration, nlevel=nlevel)

    m128 = np.zeros((N_M128, D, D), np.float32)
    m128[_M128['xp']:_M128['xp'] + 9] = _band_set(w_x, 1.0)
    m128[_M128['yp']:_M128['yp'] + 9] = _band_set(w_y, 1.0)
    m128[_M128['zp']:_M128['zp'] + 9] = _band_set(w_z, 1.0)
    m128[_M128['dp_c2']:_M128['dp_c2'] + 9] = _band_set(w_d, 1.0, consts['c2'])
    m128[_M128['dp_c2b']:_M128['dp_c2b'] + 9] = _band_set(w_d, 1.0, consts['c2b'])
    m128[_M128['Ap']:_M128['Ap'] + 9] = _band_set(wA, 1.0)
    m128[_M128['xm']:_M128['xm'] + 9] = _band_set(w_x, -1.0)
    m128[_M128['ym']:_M128['ym'] + 9] = _band_set(w_y, -1.0)
    m128[_M128['zm']:_M128['zm'] + 9] = _band_set(w_z, -1.0)
    m128[_M128['dm_c2']:_M128['dm_c2'] + 9] = _band_set(w_d, -1.0, consts['c2'])
    m128[_M128['dm_c2b']:_M128['dm_c2b'] + 9] = _band_set(w_d, -1.0, consts['c2b'])
    I = np.eye(D, dtype=np.float32)
    m128[_M128['I1']] = I
    m128[_M128['Ic1']] = consts['c1'] * I
    m128[_M128['Ic1b']] = consts['c1b'] * I
    m128[_M128['Imdiag']] = (-1.0 / diag) * I
    m128[_M128['Ipdiag']] = (1.0 / diag) * I
    m128[_M128['Iminvdt']] = (-1.0 / dt) * I

    base = dict(m128=m128,
                res0=_res_set(w_res, 128), res1=_res_set(w_res, 64),
                prol64p=_prol_mat(64).astype(np.float32),
                prol64n=(-_prol_mat(64)).astype(np.float32))
    for s in (32, 16, 8, 4, 2):
        base[f'resc{s}'] = _res_set(w_res, s)
    for s in (64, 32, 16, 8, 4, 2):
        m = np.zeros((11, s, s), np.float32)
        m[:9] = _band_set(wA, 0.0, -1.0 / diag, s, fold=False)
        m[9] = np.eye(s, dtype=np.float32)
        m[10] = np.eye(s, dtype=np.float32) / diag
        base[f'mco{s}'] = m
    for s in (1, 2, 4, 8, 16, 32):
        base[f'prol{s}'] = _prol_mat(s).astype(np.float32)

    ws = {'x': w_x, 'y': w_y, 'z': w_z, 'd': w_d}
    L12 = np.zeros((4, 2, 12, D), np.float32)
    for si, sn in enumerate(MINUS_STENCILS):
        w3 = ws[sn]
        for yc in range(2):
            dyt = 0 if yc == 0 else 2
            for dz in range(3):
                for xe in range(2):
                    m = 0 if xe == 0 else D - 1
                    xet = 0 if xe == 0 else 2
                    L12[si, yc, dz * 4 + yc * 2 + xe, m] = 2.0 * w3[dz, dyt, xet]
    base['L12'] = L12
    eI = np.zeros((D, D), np.float32)
    eI[0, 0] = 1.0
    eI[D - 1, D - 1] = 1.0
    base['edgeI'] = eI


    def shard(gf):
        out = []
        for c in range(NC):
            zmin = c * ZL - G
            idx = np.clip(np.arange(zmin, zmin + ZX), 0, D - 1)
            out.append(np.ascontiguousarray(
                np.transpose(gf[idx], (2, 0, 1)).astype(np.float32)))
        return out

    sh_u, sh_v, sh_w, sh_p, sh_s = (shard(a) for a in (vu, vv_, vw, vp, sg))

    in_maps = []
    for c in range(NC):
        is_bot, is_top = c == 0, c == NC - 1
        vmv = np.zeros(10, np.float32)
        vmv[0] = 0.0 if is_bot else 1.0
        vmv[1] = -1.0 if is_bot else 0.0
        vmv[2] = 1.0 if is_bot else 0.0
        vmv[3] = 0.0 if is_top else 1.0
        vmv[4] = -1.0 if is_top else 0.0
        vmv[5] = 1.0 if is_top else 0.0
        vmv[6] = -1.0 if is_bot else 1.0
        vmv[7] = -1.0 if is_top else 1.0
        vmv[8] = 1.0 if is_bot else -1.0
        vmv[9] = 1.0 if is_top else -1.0

        corrb = np.zeros((D, 24), np.float32)
        for si, sn in enumerate(MINUS_STENCILS):
            w3 = ws[sn]
            for side in range(2):
                mask = 1.0 if (is_bot if side == 0 else is_top) else 0.0
                dzt = 0 if side == 0 else 2
                for dy in range(3):
                    col = si * 6 + side * 3 + dy
                    corrb[0, col] = mask * 2.0 * w3[dzt, dy, 0]
                    corrb[D - 1, col] = mask * 2.0 * w3[dzt, dy, 2]

        cbv = np.zeros((D, 24), np.float32)
        for si, sn in enumerate(MINUS_STENCILS):
            w3 = ws[sn]
            for side in range(2):
                mask = 1.0 if (is_bot if side == 0 else is_top) else 0.0
                dzt = 0 if side == 0 else 2
                for dy in range(3):
                    col = si * 6 + side * 3 + dy
                    cbv[0, col] = mask * 2.0 * w3[dzt, dy, 0]
                    cbv[D - 1, col] = mask * 2.0 * w3[dzt, dy, 2]
        xs = np.arange(D, dtype=np.int32)
        bot_pl = 2 * (c - 1) + 1 if c > 0 else 0
        top_pl = 2 * (c + 1) + 0 if c < NC - 1 else 2 * c + 1
        idx_ph = np.stack([bot_pl * D + xs, top_pl * D + xs], 1).astype(np.int32)

        idx_w = np.zeros((32, 6), np.int32)
        for j, z in enumerate(range(4 * c - 1, 4 * c + 5)):
            idx_w[:, j] = (z * 32 + np.arange(32)) if 0 <= z < 32 else (32 * 32 + np.arange(32))

        m = dict(base)
        m.update(fld_u=sh_u[c], fld_v=sh_v[c], fld_w=sh_w[c], fld_p=sh_p[c],
                 fld_s=sh_s[c], vm=vmv, cb=cbv, idx_ph=idx_ph, idx_w=idx_w)
        in_maps.append(m)
    return in_maps, consts


# ------------------------------------------------------------------ builder
def _build(consts):
    global DEBUG
    dt = consts['dt']
    diag = consts['diag']
    iteration = consts['iteration']

    nc = bacc.Bacc("TRN2", target_bir_lowering=False, debug=False, num_devices=NC)
    dr = {}

    def din(name, shape, dtp=f32r):
        dr[name] = nc.dram_tensor(name, list(shape), dtp, kind="ExternalInput")

    for nm in ('fld_u', 'fld_v', 'fld_w', 'fld_p'):
        din(nm, (D, ZX, D))
    din('fld_s', (D, ZX, D), f32)
    din('m128', (N_M128, D, D))
    din('res0', (4, 128, 64), f32); din('res1', (4, 64, 32), f32)
    for s in (32, 16, 8, 4, 2):
        din(f'resc{s}', (4, s, s // 2), f32)
    for s in (64, 32, 16, 8, 4, 2):
        din(f'mco{s}', (11, s, s))
    for s in (1, 2, 4, 8, 16, 32):
        din(f'prol{s}', (s, 2 * s), f32)
    din('prol64p', (64, 128), f32); din('prol64n', (64, 128), f32)
    din('L12', (4, 2, 12, D), f32)
    din('cb', (D, 24), f32)
    din('edgeI', (D, D))
    din('vm', (10,), f32)
    din('idx_ph', (D, 2), i32); din('idx_w', (32, 6), i32)

    out_f = nc.dram_tensor("out_fields", [5, D, ZL, D], f32, kind="ExternalOutput")
    out_r = nc.dram_tensor("out_r", [1, 1], f32, kind="ExternalOutput")
    dbg = {}
    if DEBUG:
        for nm, shp in DEBUG.items():
            dbg[nm] = nc.dram_tensor("dbg_" + nm, list(shp), f32,
                                     kind="ExternalOutput")

    DMAE = [nc.sync, nc.scalar, nc.gpsimd]
    dma_i = [0]

    def dma(dst, src):
        DMAE[dma_i[0] % len(DMAE)].dma_start(dst, src)
        dma_i[0] += 1

    with tile.TileContext(nc) as tc:
        with (
            tc.tile_pool(name="pool", bufs=1) as P_,
            tc.tile_pool(name="mats", bufs=2) as MP,
            tc.tile_pool(name="tmp", bufs=2) as TP,
            tc.tile_pool(name="psum", bufs=5, space="PSUM") as PS,
            tc.tile_pool(name="psc", bufs=1, space="PSUM") as PSC,
            tc.tile_pool(name="dram", bufs=1, space="DRAM") as DP,
        ):
            r32 = lambda ap: ap.bitcast(f32)

            ids = P_.tile([D, 6, D], f32r, tag="ids")
            dma(ids[:], dr['m128'][_M128['I1']:_M128['I1'] + 6]
                .rearrange("i k m -> k i m"))

            def ID(nm):
                return ids[:, _M128[nm] - _M128['I1'], :]

            rs0 = P_.tile([128, 4, 64], f32, tag="rs0")
            dma(rs0[:], dr['res0'][:].rearrange("i k m -> k i m"))
            rs1 = P_.tile([64, 4, 32], f32, tag="rs1")
            dma(rs1[:], dr['res1'][:].rearrange("i k m -> k i m"))
            rsc, mco, prl = {}, {}, {}
            for s in (32, 16, 8, 4, 2):
                rsc[s] = P_.tile([s, 4, s // 2], f32, tag=f"rsc{s}", name=f"rsc{s}")
                dma(rsc[s][:], dr[f'resc{s}'][:].rearrange("i k m -> k i m"))
            for s in (64, 32, 16, 8, 4, 2):
                mco[s] = P_.tile([s, 11, s], f32r, tag=f"mco{s}", name=f"mco{s}")
                dma(mco[s][:], dr[f'mco{s}'][:].rearrange("i k m -> k i m"))
            for s in (1, 2, 4, 8, 16, 32):
                prl[s] = P_.tile([s, 2 * s], f32, tag=f"prl{s}", name=f"prl{s}")
                dma(prl[s][:], dr[f'prol{s}'][:])
            p64p = P_.tile([64, 128], f32, tag="p64p")
            dma(p64p[:], dr['prol64p'][:])
            p64n = P_.tile([64, 128], f32, tag="p64n")
            dma(p64n[:], dr['prol64n'][:])
            L12 = P_.tile([12, 8, D], f32, tag="L12")
            dma(L12[:], dr['L12'][:].rearrange("s c k m -> k (s c) m"))

            cb = P_.tile([D, 24], f32, tag="cb")
            dma(cb[:], dr['cb'][:])
            edgeI = P_.tile([D, D], f32r, tag="edgeI")
            dma(edgeI[:], dr['edgeI'][:])
            vm = P_.tile([D, 10], f32, tag="vm")
            dma(vm[:], dr['vm'][:].partition_broadcast(D))
            ixp = P_.tile([D, 2], i32, tag="ixp")
            dma(ixp[:], dr['idx_ph'][:])
            ixw = P_.tile([32, 6], i32, tag="ixw")
            dma(ixw[:], dr['idx_w'][:])


            class FT:
                def __init__(self, ap, z0):
                    self.ap = ap
                    self.z0 = z0
                    self.shape = ap.shape

                def __getitem__(self, k):
                    return self.ap[k]

            def ftile(tag, z0, zn):
                t = P_.tile([D, zn, YP], f32r, tag=tag, name=tag)
                return FT(t, z0)

            U = {n: ftile('t' + n, 0, ZX) for n in 'uvw'}
            p = ftile('p', 0, ZX)
            rd = P_.tile([D, ZX, D], f32, tag="rd")
            B_ = {n: ftile('b' + n, 1, 20) for n in 'uvw'}
            U2 = {n: ftile('q' + n, 2, 18) for n in 'uvw'}
            xps = {}
            for n in 'uvw':
                t = P_.tile([D, 20, D], f32, tag='xp' + n,
                            name='xp' + n)
                xps[n] = FT(t, 1)

            for n, src in (('u', 'fld_u'), ('v', 'fld_v'), ('w', 'fld_w')):
                dma(U[n][:, :, 1:129], dr[src][:])
            dma(p[:, :, 1:129], dr['fld_p'][:])
            dma(rd[:], dr['fld_s'][:])
            nc.vector.tensor_scalar(rd[:], rd[:], float(dt), 1.0,
                                    op0=ALU.mult, op1=ALU.add)
            nc.vector.reciprocal(rd[:], rd[:])
            for n in 'uvw':
                nc.vector.tensor_mul(U[n][:, :, 1:129], r32(U[n][:, :, 1:129]), rd[:])
            for (sl, vi) in ((slice(0, G), 6), (slice(ZX - G, ZX), 7)):
                nc.scalar.activation(U['u'][:, sl, 1:129], r32(U['u'][:, sl, 1:129]),
                                     AF.Copy, scale=vm[:, vi:vi + 1])

            def ycols(t, a, b_, f, gslots=()):
                la, lb = a - t.z0, b_ - t.z0
                nc.scalar.activation(t[:, la:lb, 0:1], r32(t[:, la:lb, 1:2]),
                                     AF.Copy, scale=float(f))
                nc.scalar.activation(t[:, la:lb, 129:130], r32(t[:, la:lb, 128:129]),
                                     AF.Copy, scale=float(f))
                for (g, vi) in gslots:
                    lg = g - t.z0
                    nc.scalar.activation(t[:, lg:lg + 1, 0:1],
                                         r32(t[:, lg:lg + 1, 1:2]),
                                         AF.Copy, scale=vm[:, vi:vi + 1])
                    nc.scalar.activation(t[:, lg:lg + 1, 129:130],
                                         r32(t[:, lg:lg + 1, 128:129]),
                                         AF.Copy, scale=vm[:, vi:vi + 1])

            ycols(U['u'], G, ZX - G, -1.0,
                  gslots=[(g, 8) for g in range(G)]
                  + [(g, 9) for g in range(ZX - G, ZX)])
            ycols(U['v'], 0, ZX, 1.0)
            ycols(U['w'], 0, ZX, 1.0)
            ycols(p, 0, ZX, 1.0)

            def cc3(t, o0, zout):
                c = TP.tile([12, zout], f32r, tag="cc3", bufs=1)
                for dz in range(3):
                    for yc in range(2):
                        for xe in range(2):
                            k = dz * 4 + yc * 2 + xe
                            m = 0 if xe == 0 else D - 1
                            ycol = 0 if yc == 0 else 129
                            ls = o0 - 1 + dz - t.z0
                            nc.sync.dma_start(
                                c[k:k + 1, 0:zout],
                                t[m:m + 1, ls:ls + zout, ycol:ycol + 1]
                                .rearrange("p z o -> p (z o)"))
                return c

            def mset(nm):
                t = MP.tile([D, 9, D], f32r, tag="mset")
                b0 = _M128[nm]
                dma(t[:], dr['m128'][b0:b0 + 9].rearrange("i k m -> k i m"))
                return t

            def conv_groups(T, o0, zout, mt, id_terms=(), corr=None):
                for a in range(0, zout, 4):
                    cp = min(4, zout - a)
                    ps = PS.tile([D, 4, D], f32, tag="cv")
                    ops = []
                    for dz in range(3):
                        for dy in range(3):
                            ls = o0 + a - 1 + dz - T.z0
                            ops.append((mt[:, dz * 3 + dy, :],
                                        T[:, ls:ls + cp, dy:dy + D],
                                        ps[:, 0:cp, :]))
                    for (iap, src) in id_terms:
                        lo = o0 + a - src.z0
                        ops.append((iap, src[:, lo:lo + cp, 1:129], ps[:, 0:cp, :]))
                    if corr is not None:
                        si, cct, srcT = corr
                        for yc in range(2):
                            yo = 0 if yc == 0 else D - 1
                            ops.append((L12[:, si * 2 + yc, :],
                                        cct[:, a:a + cp].bitcast(f32),
                                        ps[:, 0:cp, yo:yo + 1]))
                    n = len(ops)
                    for i, (lh, rh, o_ap) in enumerate(ops):
                        nc.tensor.matmul(o_ap, lh, rh,
                                         start=(i == 0), stop=(i == n - 1))
                    yield a, o0 + a, cp, ps

            def full_conv(T, o0, zout, mt, dst_fn, scale=1.0):
                for a, og, cp, ps in conv_groups(T, o0, zout, mt):
                    nc.scalar.activation(dst_fn(a, cp), ps[:, 0:cp, :],
                                         AF.Copy, scale=float(scale))

            # ---------------- stage 1 + 2 ----------------
            def stage_combine(comp, T, o0, zout, PROD, msets, mdf, id_terms,
                              dtc, xpsrc, dst, cc):
                is_minus = comp == 'u'
                mtile = TP.tile([D, zout, D], f32, tag="madv", bufs=1)
                for k, (sn, mul) in enumerate((('x', 'u'), ('y', 'v'), ('z', 'w'))):
                    corr = (MINUS_STENCILS.index(sn), cc, T) if is_minus else None
                    mt_k = mset(msets[sn])
                    for a, og, cp, ps in conv_groups(T, o0, zout, mt_k,
                                                     corr=corr):
                        mu = PROD[mul]
                        msl = r32(mu[:, og - mu.z0: og - mu.z0 + cp, 1:129])
                        if k == 0:
                            nc.vector.tensor_mul(mtile[:, a:a + cp, :], msl,
                                                 ps[:, 0:cp, :])
                        else:
                            tt = TP.tile([D, 4, D], f32, tag="advt", bufs=1)
                            nc.vector.tensor_mul(tt[:, 0:cp, :], msl, ps[:, 0:cp, :])
                            nc.gpsimd.tensor_add(mtile[:, a:a + cp, :],
                                                 mtile[:, a:a + cp, :], tt[:, 0:cp, :])
                corr = (3, cc, T) if is_minus else None
                mt_d = mset(mdf)
                for a, og, cp, ps in conv_groups(T, o0, zout, mt_d,
                                                 id_terms=id_terms, corr=corr):
                    nc.vector.scalar_tensor_tensor(
                        mtile[:, a:a + cp, :], mtile[:, a:a + cp, :], float(-dtc),
                        ps[:, 0:cp, :], op0=ALU.mult, op1=ALU.add)
                nc.gpsimd.tensor_add(mtile[:], mtile[:],
                                     xpsrc[:, o0 - xpsrc.z0:o0 - xpsrc.z0 + zout, :])
                nc.vector.tensor_mul(dst[:, o0 - dst.z0:o0 - dst.z0 + zout, 1:129],
                                     mtile[:], rd[:, o0:o0 + zout, :])

            for sn, n in (('xp', 'u'), ('yp', 'v'), ('zp', 'w')):
                mt = mset(sn)
                xx = xps[n]
                full_conv(p, 1, 20, mt,
                          lambda a, cp, _x=xx: _x[:, a:a + cp, :], scale=-dt)

            cc_u = cc3(U['u'], 1, 20)
            for comp in 'uvw':
                ms = ({'x': 'xm', 'y': 'ym', 'z': 'zm'} if comp == 'u'
                      else {'x': 'xp', 'y': 'yp', 'z': 'zp'})
                stage_combine(comp, U[comp], 1, 20, U, ms,
                              'dm_c2' if comp == 'u' else 'dp_c2',
                              [(ID('Ic1'), U[comp])],
                              0.5 * dt, xps[comp], B_[comp],
                              cc_u if comp == 'u' else None)

            def blend(t, slots_bot, slots_top, minus):
                eb = t[:, G - t.z0:G - t.z0 + 1, 1:129]
                et = t[:, G + ZL - 1 - t.z0:G + ZL - t.z0, 1:129]
                for (slots, edge, mvi, svi) in (
                        (slots_bot, eb, 0, 1 if minus else 2),
                        (slots_top, et, 3, 4 if minus else 5)):
                    for g in slots:
                        sl = t[:, g - t.z0:g - t.z0 + 1, 1:129]
                        nc.scalar.activation(sl, r32(sl), AF.Copy,
                                             scale=vm[:, mvi:mvi + 1])
                        nc.vector.scalar_tensor_tensor(
                            sl, r32(edge), vm[:, svi:svi + 1], r32(sl),
                            op0=ALU.mult, op1=ALU.add)

            if DEBUG:
                nc.sync.dma_start(dbg['bu'][:], r32(B_['u'][:, :, 1:129]))
                xau = TP.tile([D, 20, D], f32, tag="madv", bufs=1)
                mtl = mset('xm')
                cc_d = cc3(U['u'], 1, 20)
                for a, og, cp, ps in conv_groups(U['u'], 1, 20, mtl,
                                                 corr=(0, cc_d, U['u'])):
                    nc.scalar.activation(xau[:, a:a + cp, :], ps[:, 0:cp, :],
                                         AF.Copy)
                nc.sync.dma_start(dbg['xau'][:], xau[:])
                nc.sync.dma_start(dbg['xpu'][:], xps['u'][:])
            blend(B_['u'], (1, 2), (19, 20), True)
            blend(B_['v'], (1, 2), (19, 20), False)
            blend(B_['w'], (1, 2), (19, 20), False)
            ycols(B_['u'], G, ZX - G, -1.0,
                  gslots=[(1, 8), (2, 8), (19, 9), (20, 9)])
            ycols(B_['v'], 1, 21, 1.0)
            ycols(B_['w'], 1, 21, 1.0)

            cc_b = cc3(B_['u'], 2, 18)
            for comp in 'uvw':
                ms = ({'x': 'xm', 'y': 'ym', 'z': 'zm'} if comp == 'u'
                      else {'x': 'xp', 'y': 'yp', 'z': 'zp'})
                stage_combine(comp, B_[comp], 2, 18, B_, ms,
                              'dm_c2b' if comp == 'u' else 'dp_c2b',
                              [(ID('Ic1b'), B_[comp]), (ID('I1'), U[comp])],
                              dt, xps[comp], U2[comp],
                              cc_b if comp == 'u' else None)

            if DEBUG:
                nc.sync.dma_start(dbg['qu'][:], r32(U2['u'][:, :, 1:129]))
            blend(U2['u'], (2,), (19,), True)
            blend(U2['v'], (2,), (19,), False)
            blend(U2['w'], (2,), (19,), False)
            ycols(U2['u'], G, ZX - G, -1.0, gslots=[(2, 8), (19, 9)])
            ycols(U2['v'], 2, 20, 1.0)
            ycols(U2['w'], 2, 20, 1.0)

            # ---------------- stage 3: b ----------------
            b = FT(P_.tile([D, ZL, D], f32r, tag="bu", name="stb"), G)
            mt_x3, mt_y3, mt_z3 = mset('xm'), mset('yp'), mset('zp')
            cc_q = cc3(U2['u'], G, ZL)
            for a in range(0, ZL, 4):
                og = G + a
                ps = PS.tile([D, 4, D], f32, tag="cv")
                ops = []
                for (mt, T) in ((mt_x3, U2['u']), (mt_y3, U2['v']), (mt_z3, U2['w'])):
                    for dz in range(3):
                        for dy in range(3):
                            ls = og - 1 + dz - T.z0
                            ops.append((mt[:, dz * 3 + dy, :],
                                        T[:, ls:ls + 4, dy:dy + D], ps[:]))
                for yc in range(2):
                    yo = 0 if yc == 0 else D - 1
                    ops.append((L12[:, 0 * 2 + yc, :],
                                cc_q[:, a:a + 4].bitcast(f32),
                                ps[:, :, yo:yo + 1]))
                n = len(ops)
                for i, (lh, rh, o_ap) in enumerate(ops):
                    nc.tensor.matmul(o_ap, lh, rh, start=(i == 0), stop=(i == n - 1))
                nc.scalar.activation(b[:, a:a + 4, :], ps[:], AF.Copy,
                                     scale=float(-1.0 / dt))
                for side, ogx in ((0, G), (1, G + ZL - 1)):
                    pos = ogx - og
                    if 0 <= pos < 4:
                        gs = (G - 1 if side == 0 else ZX - G) - U2['u'].z0
                        psb = PS.tile([D, 1, D], f32, tag="cvb", name="psb",
                                      bufs=1)
                        for dy in range(3):
                            col = side * 3 + dy
                            ct = TP.tile([D, 1, D], f32r, tag="cbt", bufs=3,
                                         name="cbt")
                            nc.scalar.activation(
                                ct[:], r32(U2['u'][:, gs:gs + 1, dy:dy + D]),
                                AF.Copy, scale=cb[:, col:col + 1])
                            nc.tensor.matmul(psb[:], edgeI[:], ct[:],
                                             start=(dy == 0), stop=(dy == 2))
                        nc.vector.scalar_tensor_tensor(
                            b[:, a + pos:a + pos + 1, :], psb[:],
                            float(-1.0 / dt), r32(b[:, a + pos:a + pos + 1, :]),
                            op0=ALU.mult, op1=ALU.add)

            # ---------------- multigrid ----------------
            App = FT(P_.tile([D, ZL, D], f32r, tag="bv", name="app"), G)
            if DEBUG:
                nc.sync.dma_start(dbg['b'][:], r32(b[:]))
            wmg = P_.tile([D, ZL, D], f32, tag="bw")


            wpad = {}
            for s in (4, 8, 16, 32):
                wpad[s] = P_.tile([s, s + 2, s + 2], f32r, tag=f"wp{s}", name=f"wp{s}")
                nc.vector.memset(wpad[s][:].bitcast(f32), 0.0)
                nc.vector.tensor_copy(wpad[s][:], wpad[s][:].bitcast(f32))
            wpad[2] = P_.tile([2, 4, 4], f32r, tag="wp2", name="wp2")
            nc.vector.memset(wpad[2][:].bitcast(f32), 0.0)
            nc.vector.tensor_copy(wpad[2][:], wpad[2][:].bitcast(f32))
            w64pad = P_.tile([64, 12, 66], f32r, tag="w64p")
            nc.vector.memset(w64pad[:].bitcast(f32), 0.0)
            nc.vector.tensor_copy(w64pad[:], w64pad[:].bitcast(f32))
            w32sl = P_.tile([32, 6, 32], f32r, tag="w32sl")
            zrow32 = P_.tile([32, 32], f32, tag="zrow32")
            nc.vector.memset(zrow32[:], 0.0)
            r7_out = P_.tile([1, 1], f32, tag="r7o")

            def prol_mm(lh, src, zsl, parts, zn, width, ps, first, last):
                """prol: out[2z+a, 2y+b] = src[z, y]; 2 matmuls (a=0,1)."""
                pv = ps[:].rearrange("m (z a) y -> m a z y", a=2)
                rh = (src[:, zsl, :].bitcast(f32).unsqueeze(3)
                      .broadcast_to([parts, zn, width, 2]))
                for a in range(2):
                    nc.tensor.matmul(pv[:, a], lh, rh,
                                     start=(first and a == 0),
                                     stop=(last and a == 1))

            for it in range(iteration):
                ph_in = DP.tile([2 * D * D], f32, tag=f"phin{it}", name=f"phin{it}")
                ph_out = DP.tile([NC, 2 * D * D], f32, tag=f"phout{it}",
                                 name=f"phout{it}", addr_space="Shared")
                r2_in = DP.tile([4 * 32 * 32], f32, tag=f"r2in{it}",
                                name=f"r2in{it}")
                r2_out = DP.tile([NC, 4 * 32 * 32], f32, tag=f"r2out{it}",
                                 name=f"r2out{it}", addr_space="Shared")
                w32_d = DP.tile([33 * 32, 32], f32, tag=f"w32d{it}",
                                name=f"w32d{it}")
                apm = mset('Ap')
                full_conv(p, G, ZL, apm,
                          lambda a, cp: App[:, a:a + cp, :])
                rzero = P_.tile([D, ZL, D], f32r, tag="tv")
                nc.vector.tensor_sub(rzero[:], r32(App[:]), r32(b[:]))
                r1 = TP.tile([64, 8, 64], f32r, tag="r1", bufs=1)
                ps = PSC.tile([64, 8, 64], f32, tag="co")
                rzv = r32(rzero[:]).rearrange("k (z a) (y c) -> k a c z y", a=2, c=2)
                for i in range(4):
                    nc.tensor.matmul(ps[:], rs0[:, i, :],
                                     rzv[:, i // 2, i % 2],
                                     start=(i == 0), stop=(i == 3))
                nc.scalar.activation(r1[:], ps[:], AF.Copy)
                r2 = TP.tile([32, 4, 32], f32r, tag="r2", bufs=1)
                ps = PSC.tile([32, 4, 32], f32, tag="co")
                r1v = r32(r1[:]).rearrange("k (z a) (y c) -> k a c z y", a=2, c=2)
                for i in range(4):
                    nc.tensor.matmul(ps[:], rs1[:, i, :],
                                     r1v[:, i // 2, i % 2],
                                     start=(i == 0), stop=(i == 3))
                nc.scalar.activation(r2[:], ps[:], AF.Copy)

                nc.sync.dma_start(
                    r2_in[:].rearrange("(z x y) -> x z y", z=4, x=32, y=32),
                    r32(r2[:]))
                nc.gpsimd.collective_compute(
                    "AllGather", ALU.bypass, replica_groups=[list(range(NC))],
                    ins=[r2_in[:].opt()], outs=[r2_out[:].opt()])
                r2f = P_.tile([32, 32, 32], f32r, tag="tw", name="r2f")
                nc.sync.dma_start(
                    r2f[:],
                    r2_out[:].rearrange("c (z x y) -> x (c z) y", z=4, x=32, y=32)
                    .bitcast(f32r))

                rl = {32: r2f}
                src = r2f
                for s in (32, 16, 8, 4, 2):
                    so = s // 2
                    dstr = TP.tile([so, so, so], f32r, tag=f"rv{so}", name=f"rv{so}")
                    ps = PSC.tile([so, so, so], f32, tag="co")
                    sv = r32(src[:]).rearrange("k (z a) (y c) -> k a c z y",
                                               a=2, c=2)
                    for i in range(4):
                        nc.tensor.matmul(ps[:], rsc[s][:, i, :],
                                         sv[:, i // 2, i % 2],
                                         start=(i == 0), stop=(i == 3))
                    nc.scalar.activation(dstr[:], ps[:], AF.Copy)
                    rl[so] = dstr
                    src = dstr
                if it == iteration - 1:
                    nc.scalar.activation(r7_out[:].unsqueeze(1), r32(rl[1][:]),
                                         AF.Copy)

                wcur = TP.tile([1, 1, 1], f32r, tag="w1")
                nc.scalar.activation(wcur[:], r32(rl[1][:]), AF.Copy,
                                     scale=float(1.0 / diag))
                for s in (1, 2, 4, 8, 16):
                    s2 = 2 * s
                    nhalf = 2 if s2 == 32 else 1
                    for h in range(nhalf):
                        zh = s2 // nhalf
                        ps = PSC.tile([s2, zh, s2], f32, tag="co")
                        prol_mm(prl[s][:], wcur,
                                slice(h * zh // 2, (h + 1) * zh // 2),
                                s, zh // 2, s, ps, True, True)
                        nc.scalar.activation(
                            wpad[s2][:, 1 + h * zh:1 + (h + 1) * zh, 1:s2 + 1],
                            ps[:], AF.Copy)
                    wn = TP.tile([32, 32, 32], f32r, tag="wu", name=f"wu{s2}", bufs=1)[0:s2, 0:s2, 0:s2]
                    for h in range(nhalf):
                        zh = s2 // nhalf
                        ps2 = PSC.tile([s2, zh, s2], f32, tag="co")
                        k = 0
                        for dz in range(3):
                            for dy in range(3):
                                nc.tensor.matmul(
                                    ps2[:], mco[s2][:, dz * 3 + dy, :],
                                    wpad[s2][:, h * zh + dz:h * zh + dz + zh,
                                             dy:dy + s2],
                                    start=(k == 0), stop=False)
                                k += 1
                        nc.tensor.matmul(ps2[:], mco[s2][:, 9, :],
                                         wpad[s2][:, 1 + h * zh:1 + (h + 1) * zh,
                                                  1:s2 + 1],
                                         start=False, stop=False)
                        nc.tensor.matmul(ps2[:], mco[s2][:, 10, :],
                                         rl[s2][:, h * zh:(h + 1) * zh, :],
                                         start=False, stop=True)
                        nc.scalar.activation(wn[:, h * zh:(h + 1) * zh, :], ps2[:],
                                             AF.Copy)
                    wcur = wn

                nc.sync.dma_start(
                    w32_d[0:1024].rearrange("(z x) y -> x z y", z=32, x=32),
                    r32(wcur[:]))
                nc.sync.dma_start(w32_d[1024:1056], zrow32[:])
                nc.vector.memset(w32sl[:].bitcast(f32), 0.0)
                nc.vector.tensor_copy(w32sl[:], w32sl[:].bitcast(f32))
                for j in range(6):
                    gt = TP.tile([D, D], f32, tag="gat", name="gt32", bufs=1)[0:32, 0:32]
                    nc.gpsimd.indirect_dma_start(
                        out=gt[:], out_offset=None, in_=w32_d[:],
                        in_offset=bass.IndirectOffsetOnAxis(ap=ixw[:, j:j + 1],
                                                            axis=0))
                    nc.vector.tensor_copy(w32sl[:, j:j + 1, :], gt[:].unsqueeze(1))
                for h in range(2):
                    ps = PSC.tile([64, 6, 64], f32, tag="co")
                    prol_mm(prl[32][:], w32sl, slice(3 * h, 3 * h + 3),
                            32, 3, 32, ps, True, True)
                    nc.scalar.activation(w64pad[:, 6 * h:6 * h + 6, 1:65], ps[:],
                                         AF.Copy)
                ps2 = PSC.tile([64, 8, 64], f32, tag="co")
                k = 0
                for dz in range(3):
                    for dy in range(3):
                        nc.tensor.matmul(ps2[:], mco[64][:, dz * 3 + dy, :],
                                         w64pad[:, 1 + dz:1 + dz + 8, dy:dy + 64],
                                         start=(k == 0), stop=False)
                        k += 1
                nc.tensor.matmul(ps2[:], mco[64][:, 9, :], w64pad[:, 2:10, 1:65],
                                 start=False, stop=False)
                nc.tensor.matmul(ps2[:], mco[64][:, 10, :], r1[:],
                                 start=False, stop=True)
                w64u = TP.tile([64, 8, 64], f32r, tag="w64u", bufs=1)
                nc.scalar.activation(w64u[:], ps2[:], AF.Copy)
                if DEBUG and it == 0:
                    nc.sync.dma_start(dbg['App'][:], r32(App[:]))
                    nc.sync.dma_start(dbg['r2f'][:], r32(r2f[:]))
                    nc.sync.dma_start(dbg['w32'][:], r32(wcur[:]))
                    nc.sync.dma_start(dbg['w64u'][:], r32(w64u[:]))

                last = it == iteration - 1
                for a in range(0, ZL, 4):
                    og = G + a
                    if last:
                        psw = PS.tile([D, 4, D], f32, tag="cv")
                        prol_mm(p64p[:], w64u, slice(a // 2, a // 2 + 2),
                                64, 2, 64, psw, True, True)
                        nc.scalar.activation(wmg[:, a:a + 4, :], psw[:], AF.Copy)
                    psp = PS.tile([D, 4, D], f32, tag="cv")
                    nc.tensor.matmul(psp[:], ID('I1'), p[:, og:og + 4, 1:129],
                                     start=True, stop=False)
                    nc.tensor.matmul(psp[:], ID('Imdiag'), App[:, a:a + 4, :],
                                     start=False, stop=False)
                    nc.tensor.matmul(psp[:], ID('Ipdiag'), b[:, a:a + 4, :],
                                     start=False, stop=False)
                    prol_mm(p64n[:], w64u, slice(a // 2, a // 2 + 2),
                            64, 2, 64, psp, False, True)
                    nc.scalar.activation(p[:, og:og + 4, 1:129], psp[:], AF.Copy)

                if DEBUG and it == 0:
                    nc.sync.dma_start(dbg['p0'][:], r32(p[:, G:G + ZL, 1:129]))
                nc.sync.dma_start(
                    ph_in[0:D * D].rearrange("(x y) -> x y", x=D),
                    r32(p[:, G:G + 1, 1:129]).rearrange("x o y -> x (o y)"))
                nc.sync.dma_start(
                    ph_in[D * D:2 * D * D].rearrange("(x y) -> x y", x=D),
                    r32(p[:, G + ZL - 1:G + ZL, 1:129])
                    .rearrange("x o y -> x (o y)"))
                nc.gpsimd.collective_compute(
                    "AllGather", ALU.bypass, replica_groups=[list(range(NC))],
                    ins=[ph_in[:].opt()], outs=[ph_out[:].opt()])
                phv = ph_out[:].rearrange("c (p x y) -> (c p x) y", p=2, x=D, y=D)
                for (col, slot) in ((0, G - 1), (1, G + ZL)):
                    gt = TP.tile([D, D], f32, tag="gat", bufs=1)
                    nc.gpsimd.indirect_dma_start(
                        out=gt[:], out_offset=None, in_=phv,
                        in_offset=bass.IndirectOffsetOnAxis(
                            ap=ixp[:, col:col + 1], axis=0))
                    nc.vector.tensor_copy(p[:, slot:slot + 1, 1:129],
                                          gt[:].unsqueeze(1))
                ycols(p, G - 1, G + ZL + 1, 1.0)

            # ---------------- final correction ----------------
            for fi, comp in enumerate('uvw'):
                u3 = P_.tile([D, ZL, D], f32, tag="tu", name="u3")
                mt = mset({'u': 'xp', 'v': 'yp', 'w': 'zp'}[comp])
                for a in range(0, ZL, 4):
                    og = G + a
                    ps = PS.tile([D, 4, D], f32, tag="cv")
                    k = 0
                    for dz in range(3):
                        for dy in range(3):
                            nc.tensor.matmul(
                                ps[:], mt[:, dz * 3 + dy, :],
                                p[:, og - 1 + dz:og - 1 + dz + 4, dy:dy + D],
                                start=(k == 0), stop=False)
                            k += 1
                    qq = U2[comp]
                    nc.tensor.matmul(ps[:], ID('Iminvdt'),
                                     qq[:, og - qq.z0:og - qq.z0 + 4, 1:129],
                                     start=False, stop=True)
                    nc.vector.scalar_tensor_tensor(
                        u3[:, a:a + 4, :], ps[:], float(-dt),
                        rd[:, og:og + 4, :], op0=ALU.mult, op1=ALU.mult)
                dma(out_f[fi], u3[:])

            dma(out_f[3], r32(p[:, G:G + ZL, 1:129]))
            dma(out_f[4], wmg[:])
            nc.sync.dma_start(out_r[:], r7_out[:])

    nc.compile()
    return nc


# ------------------------------------------------------------------ entry
_CACHE = {}
DEBUG = {}


def kernel(**inputs):
    in_maps, consts = _host_prep(inputs)
    key = (consts['dt'], consts['S'], consts['diag'], consts['iteration'],
           consts['nlevel'],
           np.asarray(inputs['wA'], np.float32).tobytes(),
           np.asarray(inputs['w_res'], np.float32).tobytes())
    if key not in _CACHE:
        _CACHE[key] = _build(consts)
    res = bass_utils.run_bass_kernel_spmd(_CACHE[key], in_maps,
                                          core_ids=list(range(NC)))

    def unshard(i):
        return np.concatenate(
            [np.transpose(res.results[c]["out_fields"][i], (1, 2, 0))
             for c in range(NC)], axis=0)[None, None]

    u, v, w, pfin, wmg = (unshard(i) for i in range(5))
    r = res.results[0]["out_r"].reshape(1, 1, 1, 1, 1).astype(np.float32)
    return (u.astype(np.float32), v.astype(np.float32), w.astype(np.float32),
            pfin.astype(np.float32), wmg.astype(np.float32), r)
